# revision 35
# baseline (speedup 1.0000x reference)
"""MoE router (matmul + affine calibration + softmax + top-8) on 8 TRN2 cores.

Data-parallel over tokens: each of the 8 cores gets T/8 = 2048 tokens of the
flattened [16384, 2048] hidden_states; the (scale-folded, transposed) router
weight and the calibration bias are replicated.

Per core, per 128-token tile:
  - DMA the [128, 2048] activation tile (natural layout),
  - PE-transpose each [128,128] chunk (fp32 transpose-mode, via identity),
  - copy PSUM->SBUF (DVE/ACT), matmul-accumulate logits [128, 64] over the
    16 K-chunks with the scaled weight as the moving operand,
  - add bias, softmax (ACT Exp with accum row-sum + DVE reciprocal),
  - top-8 values + indices with DVE max8 / max_index.
"""

import numpy as np

import concourse.bass as bass
import concourse.tile as tile
from concourse import bacc, mybir
from concourse.bass_utils import run_bass_kernel_spmd
from concourse.masks import make_identity

N_CORES = 8
T = 16384          # total flattened tokens (4 * 4096)
D = 2048           # hidden dim
E = 64             # experts
TOPK = 8
P = 128            # partitions
T_LOC = T // N_CORES          # 2048 tokens per core
N_TILES = T_LOC // P          # 16 token tiles per core
N_CHUNKS = D // P             # 16 K chunks

F32 = mybir.dt.float32
U32 = mybir.dt.uint32


def _build_kernel_body(nc, tc, h, wt, cb, probs_o, topv_o, topi_o):
    import contextlib

    with contextlib.ExitStack() as ctx:
        constp = ctx.enter_context(tc.tile_pool(name="const", bufs=1))
        hin = ctx.enter_context(tc.tile_pool(name="hin", bufs=6))
        htp = ctx.enter_context(tc.tile_pool(name="ht", bufs=12))
        outp = ctx.enter_context(tc.tile_pool(name="out", bufs=3))
        smp = ctx.enter_context(tc.tile_pool(name="small", bufs=4))
        ps_tr = ctx.enter_context(tc.tile_pool(name="ps_tr", bufs=3, space="PSUM"))
        ps_lg = ctx.enter_context(tc.tile_pool(name="ps_lg", bufs=3, space="PSUM"))
        ps_b = ctx.enter_context(tc.tile_pool(name="ps_b", bufs=1, space="PSUM"))

        # ---- HAM warmup: ~4.5us of back-to-back bf16 matmuls so the PE
        # clock gate opens (cold 1.2 GHz -> warm 2.4 GHz) ----
        BF16 = mybir.dt.bfloat16
        wm_w = constp.tile([P, P], BF16)
        nc.vector.memset(wm_w, 1.0)
        wm_x = constp.tile([P, 512], BF16)
        nc.vector.memset(wm_x, 1.0)
        for wi in range(12):
            wm_ps = ps_b.tile([P, 512], F32, tag="warm")
            nc.tensor.matmul(wm_ps[:], lhsT=wm_w[:], rhs=wm_x[:],
                             start=True, stop=True)

        # ---- constants ----
        ident = constp.tile([P, P], F32)
        make_identity(nc, ident)

        wt_sb = constp.tile([P, N_CHUNKS, E], F32)  # wT chunks: [p, c, e]
        nc.sync.dma_start(wt_sb[:], wt.rearrange("(c p) e -> p c e", p=P))

        ones_col = constp.tile([1, P], F32)
        nc.vector.memset(ones_col, 1.0)
        bias_sb = constp.tile([1, E], F32)
        nc.sync.dma_start(bias_sb[:], cb[:])
        # broadcast bias to all 128 partitions: ones.T @ bias
        bias_ps = ps_b.tile([P, E], F32)
        nc.tensor.matmul(bias_ps[:], lhsT=ones_col[:], rhs=bias_sb[:],
                         start=True, stop=True)
        bias_bc = constp.tile([P, E], F32)
        nc.vector.tensor_copy(bias_bc[:], bias_ps[:])

        # accumulate outputs in SBUF; one batched DMA per output at the end
        probs_acc = constp.tile([P, N_TILES, E], F32, tag="probs_acc")
        topv_acc = constp.tile([P, N_TILES, TOPK], F32, tag="topv_acc")
        topi_acc = constp.tile([P, N_TILES, TOPK], U32, tag="topi_acc")

        # ---- main loop over token tiles ----
        for i in range(N_TILES):
            h_nat = hin.tile([P, D], F32)
            nc.sync.dma_start(h_nat[:], h[i * P:(i + 1) * P, :])

            lg_ps = ps_lg.tile([P, E], F32)
            for c in range(N_CHUNKS):
                tr_ps = ps_tr.tile([P, P], F32)
                nc.tensor.transpose(tr_ps[:], h_nat[:, c * P:(c + 1) * P],
                                    ident[:])
                ht = htp.tile([P, P], F32)
                nc.any.tensor_copy(ht[:], tr_ps[:])
                nc.tensor.matmul(lg_ps[:], lhsT=ht[:], rhs=wt_sb[:, c, :],
                                 start=(c == 0), stop=(c == N_CHUNKS - 1))

            # epilogue: bias, softmax, top-8
            lg = outp.tile([P, E], F32)
            nc.vector.tensor_add(lg[:], lg_ps[:], bias_bc[:])

            negmax = smp.tile([P, 1], F32)
            nc.vector.reduce_max(negmax[:], lg[:], axis=mybir.AxisListType.X,
                                 negate=True)

            probs = probs_acc[:, i, :]
            sumexp = smp.tile([P, 1], F32)
            nc.scalar.activation(probs[:], lg[:],
                                 mybir.ActivationFunctionType.Exp,
                                 bias=negmax[:, 0:1], scale=1.0,
                                 accum_out=sumexp[:, 0:1])
            recip = smp.tile([P, 1], F32)
            nc.vector.reciprocal(recip[:], sumexp[:])
            nc.vector.tensor_scalar_mul(probs[:], probs[:], recip[:, 0:1])

            topv = topv_acc[:, i, :]
            nc.vector.max(topv[:], probs[:])
            topi = topi_acc[:, i, :]
            nc.vector.max_index(topi[:], topv[:], probs[:])

        nc.gpsimd.dma_start(probs_o.rearrange("(i p) e -> p i e", p=P),
                            probs_acc[:])
        nc.gpsimd.dma_start(topv_o.rearrange("(i p) k -> p i k", p=P),
                            topv_acc[:])
        nc.gpsimd.dma_start(topi_o.rearrange("(i p) k -> p i k", p=P),
                            topi_acc[:])


_NC_CACHE = None


def build_nc():
    global _NC_CACHE
    if _NC_CACHE is not None:
        return _NC_CACHE
    nc = bacc.Bacc("TRN2", target_bir_lowering=False, debug=False,
                   num_devices=N_CORES)
    h = nc.dram_tensor("h", [T_LOC, D], F32, kind="ExternalInput").ap()
    wt = nc.dram_tensor("wt", [D, E], F32, kind="ExternalInput").ap()
    cb = nc.dram_tensor("cb", [1, E], F32, kind="ExternalInput").ap()
    probs_o = nc.dram_tensor("probs", [T_LOC, E], F32,
                             kind="ExternalOutput").ap()
    topv_o = nc.dram_tensor("topv", [T_LOC, TOPK], F32,
                            kind="ExternalOutput").ap()
    topi_o = nc.dram_tensor("topi", [T_LOC, TOPK], U32,
                            kind="ExternalOutput").ap()
    with tile.TileContext(nc) as tc:
        _build_kernel_body(nc, tc, h, wt, cb, probs_o, topv_o, topi_o)
    nc.compile()
    _NC_CACHE = nc
    return nc


def _prep_inputs(hidden_states, router_weight, cal_scale, cal_bias):
    h2d = np.ascontiguousarray(
        hidden_states.reshape(-1, D).astype(np.float32, copy=False))
    # fold the per-expert calibration scale into the (transposed) weight
    wt = np.ascontiguousarray(
        (router_weight.astype(np.float32) * cal_scale.astype(np.float32)[:, None]).T)
    cb = np.ascontiguousarray(cal_bias.astype(np.float32).reshape(1, E))
    in_maps = []
    for i in range(N_CORES):
        in_maps.append({
            "h": np.ascontiguousarray(h2d[i * T_LOC:(i + 1) * T_LOC]),
            "wt": wt,
            "cb": cb,
        })
    return in_maps


def run(hidden_states, router_weight, cal_scale, cal_bias, **run_kwargs):
    nc = build_nc()
    in_maps = _prep_inputs(hidden_states, router_weight, cal_scale, cal_bias)
    res = run_bass_kernel_spmd(nc, in_maps, core_ids=list(range(N_CORES)),
                               **run_kwargs)
    probs = np.concatenate([res.results[i]["probs"] for i in range(N_CORES)])
    topv = np.concatenate([res.results[i]["topv"] for i in range(N_CORES)])
    topi = np.concatenate([res.results[i]["topi"] for i in range(N_CORES)])
    out = (probs.astype(np.float32),
           topv.astype(np.float32),
           topi.view(np.int32).astype(np.int32))
    return out, res


def kernel(hidden_states, router_weight, cal_scale, cal_bias):
    out, _ = run(hidden_states, router_weight, cal_scale, cal_bias)
    return out


# revision 36
# speedup vs baseline: 1.0957x; 1.0957x over previous
"""MoE router (matmul + affine calibration + softmax + top-8) on 8 TRN2 cores.

Data-parallel over tokens: each of the 8 cores gets T/8 = 2048 tokens of the
flattened [16384, 2048] hidden_states; the (scale-folded, transposed) router
weight and the calibration bias are replicated.

Per core, per 128-token tile:
  - DMA the [128, 2048] activation tile (natural layout),
  - PE-transpose each [128,128] chunk (fp32 transpose-mode, via identity),
  - copy PSUM->SBUF (DVE/ACT), matmul-accumulate logits [128, 64] over the
    16 K-chunks with the scaled weight as the moving operand,
  - add bias, softmax (ACT Exp with accum row-sum + DVE reciprocal),
  - top-8 values + indices with DVE max8 / max_index.
"""

import numpy as np

import concourse.bass as bass
import concourse.tile as tile
from concourse import bacc, mybir
from concourse.bass_utils import run_bass_kernel_spmd
from concourse.masks import make_identity

N_CORES = 8
T = 16384          # total flattened tokens (4 * 4096)
D = 2048           # hidden dim
E = 64             # experts
TOPK = 8
P = 128            # partitions
T_LOC = T // N_CORES          # 2048 tokens per core
N_TILES = T_LOC // P          # 16 token tiles per core
N_CHUNKS = D // P             # 16 K chunks

F32 = mybir.dt.float32
U32 = mybir.dt.uint32


def _build_kernel_body(nc, tc, h, wt, cb, probs_o, topv_o, topi_o):
    import contextlib

    with contextlib.ExitStack() as ctx:
        constp = ctx.enter_context(tc.tile_pool(name="const", bufs=1))
        hin = ctx.enter_context(tc.tile_pool(name="hin", bufs=4))
        htp = ctx.enter_context(tc.tile_pool(name="ht", bufs=8))
        outp = ctx.enter_context(tc.tile_pool(name="out", bufs=3))
        smp = ctx.enter_context(tc.tile_pool(name="small", bufs=4))
        ps_tr = ctx.enter_context(tc.tile_pool(name="ps_tr", bufs=4, space="PSUM"))
        ps_lg = ctx.enter_context(tc.tile_pool(name="ps_lg", bufs=2, space="PSUM"))
        ps_b = ctx.enter_context(tc.tile_pool(name="ps_b", bufs=1, space="PSUM"))

        # ---- HAM warmup: ~4.5us of back-to-back bf16 matmuls so the PE
        # clock gate opens (cold 1.2 GHz -> warm 2.4 GHz) ----
        BF16 = mybir.dt.bfloat16
        wm_w = constp.tile([P, P], BF16)
        nc.vector.memset(wm_w, 1.0)
        wm_x = constp.tile([P, 512], BF16)
        nc.vector.memset(wm_x, 1.0)
        for wi in range(12):
            wm_ps = ps_b.tile([P, 512], F32, tag="warm")
            nc.tensor.matmul(wm_ps[:], lhsT=wm_w[:], rhs=wm_x[:],
                             start=True, stop=True)

        # ---- constants ----
        ident = constp.tile([P, P], F32)
        make_identity(nc, ident)

        wt_sb = constp.tile([P, N_CHUNKS, E], F32)  # wT chunks: [p, c, e]
        nc.sync.dma_start(wt_sb[:], wt.rearrange("(c p) e -> p c e", p=P))

        ones_col = constp.tile([1, P], F32)
        nc.vector.memset(ones_col, 1.0)
        bias_sb = constp.tile([1, E], F32)
        nc.sync.dma_start(bias_sb[:], cb[:])
        # broadcast bias to all 128 partitions: ones.T @ bias
        bias_ps = ps_b.tile([P, E], F32)
        nc.tensor.matmul(bias_ps[:], lhsT=ones_col[:], rhs=bias_sb[:],
                         start=True, stop=True)
        bias_bc = constp.tile([P, E], F32)
        nc.vector.tensor_copy(bias_bc[:], bias_ps[:])

        # accumulate outputs in SBUF; one batched DMA per output at the end
        probs_acc = constp.tile([P, N_TILES, E], F32, tag="probs_acc")
        topv_acc = constp.tile([P, N_TILES, TOPK], F32, tag="topv_acc")
        topi_acc = constp.tile([P, N_TILES, TOPK], U32, tag="topi_acc")

        # ---- main loop over token tiles ----
        for i in range(N_TILES):
            h_nat = hin.tile([P, D], F32)
            nc.sync.dma_start(h_nat[:], h[i * P:(i + 1) * P, :])

            lg_ps = ps_lg.tile([P, E], F32)
            for c in range(N_CHUNKS):
                tr_ps = ps_tr.tile([P, P], F32)
                nc.tensor.transpose(tr_ps[:], h_nat[:, c * P:(c + 1) * P],
                                    ident[:])
                ht = htp.tile([P, P], F32)
                nc.any.tensor_copy(ht[:], tr_ps[:])
                nc.tensor.matmul(lg_ps[:], lhsT=ht[:], rhs=wt_sb[:, c, :],
                                 start=(c == 0), stop=(c == N_CHUNKS - 1))

            # epilogue: bias, softmax, top-8
            lg = outp.tile([P, E], F32)
            nc.vector.tensor_add(lg[:], lg_ps[:], bias_bc[:])

            negmax = smp.tile([P, 1], F32)
            nc.vector.reduce_max(negmax[:], lg[:], axis=mybir.AxisListType.X,
                                 negate=True)

            probs = probs_acc[:, i, :]
            sumexp = smp.tile([P, 1], F32)
            nc.scalar.activation(probs[:], lg[:],
                                 mybir.ActivationFunctionType.Exp,
                                 bias=negmax[:, 0:1], scale=1.0,
                                 accum_out=sumexp[:, 0:1])
            recip = smp.tile([P, 1], F32)
            nc.vector.reciprocal(recip[:], sumexp[:])
            nc.vector.tensor_scalar_mul(probs[:], probs[:], recip[:, 0:1])

            topv = topv_acc[:, i, :]
            nc.vector.max(topv[:], probs[:])
            topi = topi_acc[:, i, :]
            nc.vector.max_index(topi[:], topv[:], probs[:])

        nc.gpsimd.dma_start(probs_o.rearrange("(i p) e -> p i e", p=P),
                            probs_acc[:])
        nc.gpsimd.dma_start(topv_o.rearrange("(i p) k -> p i k", p=P),
                            topv_acc[:])
        nc.gpsimd.dma_start(topi_o.rearrange("(i p) k -> p i k", p=P),
                            topi_acc[:])


_NC_CACHE = None


def build_nc():
    global _NC_CACHE
    if _NC_CACHE is not None:
        return _NC_CACHE
    nc = bacc.Bacc("TRN2", target_bir_lowering=False, debug=False,
                   num_devices=N_CORES)
    h = nc.dram_tensor("h", [T_LOC, D], F32, kind="ExternalInput").ap()
    wt = nc.dram_tensor("wt", [D, E], F32, kind="ExternalInput").ap()
    cb = nc.dram_tensor("cb", [1, E], F32, kind="ExternalInput").ap()
    probs_o = nc.dram_tensor("probs", [T_LOC, E], F32,
                             kind="ExternalOutput").ap()
    topv_o = nc.dram_tensor("topv", [T_LOC, TOPK], F32,
                            kind="ExternalOutput").ap()
    topi_o = nc.dram_tensor("topi", [T_LOC, TOPK], U32,
                            kind="ExternalOutput").ap()
    with tile.TileContext(nc) as tc:
        _build_kernel_body(nc, tc, h, wt, cb, probs_o, topv_o, topi_o)
    nc.compile()
    _NC_CACHE = nc
    return nc


def _prep_inputs(hidden_states, router_weight, cal_scale, cal_bias):
    h2d = np.ascontiguousarray(
        hidden_states.reshape(-1, D).astype(np.float32, copy=False))
    # fold the per-expert calibration scale into the (transposed) weight
    wt = np.ascontiguousarray(
        (router_weight.astype(np.float32) * cal_scale.astype(np.float32)[:, None]).T)
    cb = np.ascontiguousarray(cal_bias.astype(np.float32).reshape(1, E))
    in_maps = []
    for i in range(N_CORES):
        in_maps.append({
            "h": np.ascontiguousarray(h2d[i * T_LOC:(i + 1) * T_LOC]),
            "wt": wt,
            "cb": cb,
        })
    return in_maps


def run(hidden_states, router_weight, cal_scale, cal_bias, **run_kwargs):
    nc = build_nc()
    in_maps = _prep_inputs(hidden_states, router_weight, cal_scale, cal_bias)
    res = run_bass_kernel_spmd(nc, in_maps, core_ids=list(range(N_CORES)),
                               **run_kwargs)
    probs = np.concatenate([res.results[i]["probs"] for i in range(N_CORES)])
    topv = np.concatenate([res.results[i]["topv"] for i in range(N_CORES)])
    topi = np.concatenate([res.results[i]["topi"] for i in range(N_CORES)])
    out = (probs.astype(np.float32),
           topv.astype(np.float32),
           topi.view(np.int32).astype(np.int32))
    return out, res


def kernel(hidden_states, router_weight, cal_scale, cal_bias):
    out, _ = run(hidden_states, router_weight, cal_scale, cal_bias)
    return out
